# revision 5
# baseline (speedup 1.0000x reference)
"""Trainium2 Bass kernel for the LSTM seq2seq autoencoder.

Strategy:
  - Data-parallel over batch: B=512 -> 64 rows per core on 8 cores.
  - Layout A on-chip: batch on partitions (64), features on free dim.
  - All transposes of the *data* (input, output) are done on HOST numpy:
    device streams x^T tiles and emits y^T tiles.
  - Encoder length masking:
      c is frozen exactly by forcing gate preactivations (i -> -BIG,
      f -> +BIG) through an extra (mbar_t x FREEZE) rank-1 matmul row
      packed into the per-step lhsT; h is frozen with a 3-op masked blend.
  - Decoder feedback y_{t-1} @ Wih.T is algebraically folded into the
    recurrence: W_comb = Whh + Wih_dec @ out_W, so the autoregressive
    chain is a single K=256 matmul per step; y itself is computed off the
    critical path purely for output.
  - Gate order permuted to [i, f, o, g] so one sigmoid covers i,f,o.
"""

import numpy as np
from contextlib import ExitStack

import concourse.bass as bass
import concourse.bacc as bacc
import concourse.mybir as mybir
import concourse.tile as tile
from concourse.bass_utils import run_bass_kernel_spmd

B, T, D, H = 512, 512, 64, 256
G4 = 4 * H  # 1024
NCORES = 8
BL = B // NCORES  # 64
TDEC = T - 1      # 511 decoder steps
BIG = 30000.0
F32 = mybir.dt.float32

_PROGRAM = None


def _gate_perm():
    # torch gate order i,f,g,o -> ours i,f,o,g
    r = np.arange(H)
    return np.concatenate([r, H + r, 3 * H + r, 2 * H + r])


def build_program(t_enc=T, t_dec=TDEC):
    nc = bacc.Bacc(None, target_bir_lowering=False)
    f = F32
    xp_d = nc.dram_tensor("xp", [t_enc, 66, BL], f, kind="ExternalInput")
    x0p_d = nc.dram_tensor("x0p", [65, BL], f, kind="ExternalInput")
    wxenc_d = nc.dram_tensor("wxenc", [66, G4], f, kind="ExternalInput")
    whhenc_d = nc.dram_tensor("whhenc", [128, 2, G4], f, kind="ExternalInput")
    whhdec_d = nc.dram_tensor("whhdec", [128, 2, G4], f, kind="ExternalInput")
    wcomb_d = nc.dram_tensor("wcomb", [128, 2, G4], f, kind="ExternalInput")
    wxdec_d = nc.dram_tensor("wxdec", [65, G4], f, kind="ExternalInput")
    bcomb_d = nc.dram_tensor("bcomb", [1, G4], f, kind="ExternalInput")
    outw_d = nc.dram_tensor("outw", [128, 2, D], f, kind="ExternalInput")
    outb_d = nc.dram_tensor("outb", [1, D], f, kind="ExternalInput")
    outbc_d = nc.dram_tensor("outbc", [D, 1], f, kind="ExternalInput")
    masks_d = nc.dram_tensor("masks", [BL, 2, t_enc], f, kind="ExternalInput")
    ident_d = nc.dram_tensor("ident", [64, 64], f, kind="ExternalInput")
    yt_d = nc.dram_tensor("yt", [t_dec + 1, D, BL], f, kind="ExternalOutput")

    Sig = mybir.ActivationFunctionType.Sigmoid
    Tanh = mybir.ActivationFunctionType.Tanh

    with ExitStack() as ctx:
        tc = ctx.enter_context(tile.TileContext(nc))
        singles = ctx.enter_context(tc.tile_pool(name="singles", bufs=1))
        xpool = ctx.enter_context(tc.tile_pool(name="xpool", bufs=6))
        work = ctx.enter_context(tc.tile_pool(name="work", bufs=3))
        hpool = ctx.enter_context(tc.tile_pool(name="hpool", bufs=2))
        cpool = ctx.enter_context(tc.tile_pool(name="cpool", bufs=2))
        htp = ctx.enter_context(tc.tile_pool(name="htp", bufs=2))
        gpool = ctx.enter_context(
            tc.tile_pool(name="gpool", bufs=2, space=bass.MemorySpace.PSUM))
        tpp = ctx.enter_context(
            tc.tile_pool(name="tpp", bufs=2, space=bass.MemorySpace.PSUM))
        ypool = ctx.enter_context(
            tc.tile_pool(name="ypool", bufs=2, space=bass.MemorySpace.PSUM))

        # ---- persistent constants ----
        s_wxenc = singles.tile([66, G4], f)
        nc.sync.dma_start(s_wxenc, wxenc_d[:, :])
        s_whhenc = singles.tile([128, 2, G4], f)
        nc.sync.dma_start(s_whhenc, whhenc_d[:, :, :])
        s_whhdec = singles.tile([128, 2, G4], f)
        nc.sync.dma_start(s_whhdec, whhdec_d[:, :, :])
        s_wcomb = singles.tile([128, 2, G4], f)
        nc.sync.dma_start(s_wcomb, wcomb_d[:, :, :])
        s_wxdec = singles.tile([65, G4], f)
        nc.sync.dma_start(s_wxdec, wxdec_d[:, :])
        s_bcomb = singles.tile([1, G4], f)
        nc.sync.dma_start(s_bcomb, bcomb_d[:, :])
        s_outw = singles.tile([128, 2, D], f)
        nc.sync.dma_start(s_outw, outw_d[:, :, :])
        s_outb = singles.tile([1, D], f)
        nc.sync.dma_start(s_outb, outb_d[:, :])
        s_masks = singles.tile([BL, 2, t_enc], f)
        nc.sync.dma_start(s_masks, masks_d[:, :, :])
        s_ident0 = singles.tile([64, 64], f)
        nc.sync.dma_start(s_ident0, ident_d[:, :])
        s_x0p0 = singles.tile([65, BL], f)
        nc.sync.dma_start(s_x0p0, x0p_d[:, :])
        s_outbc = singles.tile([D, 1], f)
        nc.sync.dma_start(s_outbc, outbc_d[:, :])
        s_ones = singles.tile([1, BL], f)
        nc.vector.memset(s_ones, 1.0)
        # route first-touch deps of matmul operands through DVE (one sem)
        s_ident = singles.tile([64, 64], f, tag="identv")
        nc.vector.tensor_copy(s_ident, s_ident0)
        s_x0p = singles.tile([65, BL], f, tag="x0pv")
        nc.vector.tensor_copy(s_x0p, s_x0p0)
        s_bcomb0 = s_bcomb
        s_bcomb = singles.tile([1, G4], f, tag="bcombv")
        nc.vector.tensor_copy(s_bcomb, s_bcomb0)
        s_outw0 = s_outw
        s_outw = singles.tile([128, 2, D], f, tag="outwv")
        nc.vector.tensor_copy(s_outw, s_outw0)

        # ---- initial state ----
        h_prev = singles.tile([BL, H], f, tag="h0")
        nc.vector.memset(h_prev, 0.0)
        c_prev = singles.tile([BL, H], f, tag="c0")
        nc.vector.memset(c_prev, 0.0)
        hT_prev = singles.tile([128, 2, BL], f, tag="ht0")
        nc.vector.memset(hT_prev, 0.0)

        def cell_tail(ps, masked_t):
            """PSUM gates [BL, 1024] (order i,f,o,g) -> returns (h_new, c_new).
            masked_t: encoder step index for h-masking, or None (decoder)."""
            nonlocal h_prev, c_prev
            s_ifo = work.tile([BL, 3 * H], f, tag="sifo")
            nc.scalar.activation(s_ifo, ps[:, 0:3 * H], Sig)
            g_t = work.tile([BL, H], f, tag="gt")
            nc.scalar.activation(g_t, ps[:, 3 * H:G4], Tanh)
            fc = work.tile([BL, H], f, tag="fc")
            nc.vector.tensor_mul(fc, s_ifo[:, H:2 * H], c_prev)
            ig = work.tile([BL, H], f, tag="ig")
            nc.vector.tensor_mul(ig, s_ifo[:, 0:H], g_t)
            c_new = cpool.tile([BL, H], f, tag="c")
            nc.vector.tensor_add(c_new, fc, ig)
            tc_t = work.tile([BL, H], f, tag="tct")
            nc.scalar.activation(tc_t, c_new, Tanh)
            if masked_t is None:
                h_new = hpool.tile([BL, H], f, tag="h")
                nc.vector.tensor_mul(h_new, s_ifo[:, 2 * H:3 * H], tc_t)
            else:
                h2 = work.tile([BL, H], f, tag="h2")
                nc.vector.tensor_mul(h2, s_ifo[:, 2 * H:3 * H], tc_t)
                hm = work.tile([BL, H], f, tag="hm")
                nc.vector.tensor_scalar_mul(
                    hm, h2, s_masks[:, 0, masked_t:masked_t + 1])
                hp = work.tile([BL, H], f, tag="hp")
                nc.vector.tensor_scalar_mul(
                    hp, h_prev, s_masks[:, 1, masked_t:masked_t + 1])
                h_new = hpool.tile([BL, H], f, tag="h")
                nc.vector.tensor_add(h_new, hm, hp)
            h_prev = h_new
            c_prev = c_new
            return h_new

        def transpose_h(h_new):
            nonlocal hT_prev
            hT = htp.tile([128, 2, BL], f, tag="hT")
            tp = tpp.tile([128, 2, BL], f, tag="tp")
            for kc in range(2):
                nc.tensor.transpose(tp[:, kc, :],
                                    h_new[:, kc * 128:(kc + 1) * 128],
                                    s_ident)
            nc.vector.tensor_copy(hT, tp)
            hT_prev = hT

        # ================= ENCODER =================
        for t in range(t_enc):
            xp_t = xpool.tile([66, BL], f, tag="xp")
            nc.sync.dma_start(xp_t, xp_d[t, :, :])
            ps = gpool.tile([BL, G4], f, tag="g")
            for nb in range(2):
                sl = slice(nb * 512, (nb + 1) * 512)
                nc.tensor.matmul(ps[:, sl], hT_prev[:, 0, :],
                                 s_whhenc[:, 0, sl], start=True, stop=False)
            for nb in range(2):
                sl = slice(nb * 512, (nb + 1) * 512)
                nc.tensor.matmul(ps[:, sl], xp_t, s_wxenc[:, sl],
                                 start=False, stop=False)
            for nb in range(2):
                sl = slice(nb * 512, (nb + 1) * 512)
                nc.tensor.matmul(ps[:, sl], hT_prev[:, 1, :],
                                 s_whhenc[:, 1, sl], start=False, stop=True)
            h_new = cell_tail(ps, t)
            transpose_h(h_new)

        # ================= DECODER =================
        for j in range(t_dec):
            ps = gpool.tile([BL, G4], f, tag="g")
            if j == 0:
                for nb in range(2):
                    sl = slice(nb * 512, (nb + 1) * 512)
                    nc.tensor.matmul(ps[:, sl], s_x0p, s_wxdec[:, sl],
                                     start=True, stop=False)
                whh = s_whhdec
            else:
                for nb in range(2):
                    sl = slice(nb * 512, (nb + 1) * 512)
                    nc.tensor.matmul(ps[:, sl], s_ones, s_bcomb[:, sl],
                                     start=True, stop=False)
                whh = s_wcomb
            for kc in range(2):
                for nb in range(2):
                    sl = slice(nb * 512, (nb + 1) * 512)
                    nc.tensor.matmul(ps[:, sl], hT_prev[:, kc, :],
                                     whh[:, kc, sl],
                                     start=False, stop=(kc == 1))
            h_new = cell_tail(ps, None)
            transpose_h(h_new)
            # y_j -> output slot j+1 : y^T[d,b] = out_b[d] + sum_k outWT[k,d] hT[k,b]
            yps = ypool.tile([D, BL], f, tag="y")
            for kc in range(2):
                nc.tensor.matmul(yps, s_outw[:, kc, :], hT_prev[:, kc, :],
                                 start=(kc == 0), stop=(kc == 1))
            y_sb = work.tile([D, BL], f, tag="ysb")
            nc.vector.tensor_scalar_add(y_sb, yps, s_outbc)
            nc.sync.dma_start(yt_d[j + 1, :, :], y_sb)

    nc.compile()
    return nc


def _prep_host(inputs, t_enc=T, t_dec=TDEC):
    """Build per-core in_maps from full inputs (numpy, all fp32)."""
    perm = _gate_perm()
    x = np.asarray(inputs["input_tensor"], np.float32)
    tgt = np.asarray(inputs["target_tensor"], np.float32)
    lens = np.asarray(inputs["lens"]).astype(np.int64)

    eWih = np.asarray(inputs["enc_Wih"], np.float32)[perm]
    eWhh = np.asarray(inputs["enc_Whh"], np.float32)[perm]
    eb = (np.asarray(inputs["enc_bih"], np.float32)
          + np.asarray(inputs["enc_bhh"], np.float32))[perm]
    dWih = np.asarray(inputs["dec_Wih"], np.float32)[perm]
    dWhh = np.asarray(inputs["dec_Whh"], np.float32)[perm]
    db = (np.asarray(inputs["dec_bih"], np.float32)
          + np.asarray(inputs["dec_bhh"], np.float32))[perm]
    oW = np.asarray(inputs["out_W"], np.float32)
    ob = np.asarray(inputs["out_b"], np.float32)

    freeze = np.zeros(G4, np.float32)
    freeze[0:H] = -BIG      # i -> 0
    freeze[H:2 * H] = BIG   # f -> 1

    wxenc = np.concatenate([eWih.T, eb[None, :], freeze[None, :]], 0)  # [66,G4]
    whhencT = eWhh.T.reshape(2, 128, G4).transpose(1, 0, 2).copy()     # [128,2,G4]
    whhdecT = dWhh.T.reshape(2, 128, G4).transpose(1, 0, 2).copy()
    wcomb = dWhh + dWih @ oW                                           # [G4,H]
    wcombT = wcomb.T.reshape(2, 128, G4).transpose(1, 0, 2).copy()
    bcomb = (db + dWih @ ob)[None, :]                                  # [1,G4]
    wxdec = np.concatenate([dWih.T, db[None, :]], 0)                   # [65,G4]
    outwT = oW.T.reshape(2, 128, D).transpose(1, 0, 2).copy()          # [128,2,D]
    outb = ob[None, :]
    ident = np.eye(64, dtype=np.float32)

    tt = np.arange(t_enc)[None, :]
    in_maps = []
    for c in range(NCORES):
        b0 = c * BL
        xs = x[b0:b0 + BL, :t_enc, :]                # [BL,t,D]
        xp = np.empty((t_enc, 66, BL), np.float32)
        xp[:, 0:D, :] = xs.transpose(1, 2, 0)
        xp[:, D, :] = 1.0
        lc = lens[b0:b0 + BL]
        mbar = (tt >= lc[:, None]).astype(np.float32)  # [BL,t]
        xp[:, D + 1, :] = mbar.T
        x0p = np.empty((65, BL), np.float32)
        x0p[0:D, :] = tgt[b0:b0 + BL, 0, :].T
        x0p[D, :] = 1.0
        masks = np.stack([1.0 - mbar, mbar], 1)        # [BL,2,t]
        in_maps.append({
            "xp": np.ascontiguousarray(xp),
            "x0p": x0p,
            "wxenc": wxenc, "whhenc": whhencT, "whhdec": whhdecT,
            "wcomb": wcombT, "wxdec": wxdec, "bcomb": bcomb,
            "outw": outwT, "outb": outb, "outbc": ob[:, None].copy(),
            "masks": np.ascontiguousarray(masks),
            "ident": ident,
        })
    return in_maps, lens


def kernel(**inputs) -> np.ndarray:
    global _PROGRAM
    if _PROGRAM is None:
        _PROGRAM = build_program()
    nc = _PROGRAM
    in_maps, lens = _prep_host(inputs)
    res = run_bass_kernel_spmd(nc, in_maps, core_ids=list(range(NCORES)))
    out = np.zeros((B, T, D), np.float32)
    for c in range(NCORES):
        yt = res.results[c]["yt"]                      # [T, D, BL]
        out[c * BL:(c + 1) * BL] = yt.transpose(2, 0, 1)
    mask = (np.arange(T)[None, :] < lens[:, None])[:, :, None]
    out *= mask
    out[:, 0, :] = 0.0
    return out
